# revision 34
# baseline (speedup 1.0000x reference)
"""GPT-2 style transformer block on 8 TRN2 NeuronCores.

Sharding: token-data-parallel. Each batch's 2048 tokens are split into 8
chunks of 256; core c owns batch c//4 and chunks {j, 7-j} (j = c%4) so
causal attention work is balanced. QKV/proj/MLP/LN are purely local; the
only collectives are two AllGathers (k^T, then v) within each 4-core
batch group. Causality is enforced with per-core 0/1 mask tensors so all
cores run one identical SPMD graph (uniform loop bounds; masks zero the
beyond-causal tiles, which also makes the per-core graphs j-independent).

QKV and proj matmuls run in fp8e4m3 DoubleRow mode (256-deep
contraction per pass, 2x bf16 throughput); scores run in plain fp8e4m3
(k arrives fp8 from the collective, q is written fp8 — no cast); the
MLP stays bf16 (fp8 activations there exceed the error budget).
LN/softmax/residuals in f32; attention av in bf16. LN affine params are
folded into the following matmul weights host-side; the attention
1/sqrt(hd) scale is folded into w_q; the v-bias is folded into the proj
bias via the softmax-rows-sum-to-one identity. Softmax is computed
without max-subtraction as exp(s) normalized by a denominator obtained
for free as an extra ones-column in the av matmul. v is exchanged in
token-half order so av for key tiles 0..7 can start after the first v
AllGather; a tiny AllGather at t=0 absorbs cross-core launch skew; MLP
weights are SBUF-resident, DMA'd behind the QKV weight loads.
"""

import os
import sys

sys.path.insert(0, "/opt/trn_rl_repo")

import numpy as np
import ml_dtypes

import concourse.bass as bass
import concourse.tile as tile
from concourse import bacc, mybir
from concourse.bass_utils import run_bass_kernel_spmd
from concourse.masks import make_identity
from concourse.tile import add_dep_helper

F32 = mybir.dt.float32
FP8 = mybir.dt.float8e4
FP8E5 = mybir.dt.float8e5
BF16 = mybir.dt.bfloat16
BF = ml_dtypes.bfloat16
E4 = ml_dtypes.float8_e4m3
E5 = ml_dtypes.float8_e5m2
DR = mybir.MatmulPerfMode.DoubleRow

B, T, C, H, HD = 2, 2048, 768, 12, 64
EPS = 1e-5
NCORES = 8
CHUNK = 256            # global chunk size (tokens)
TLOC = 512             # local tokens per core (2 chunks)
NKT = T // 128         # 16 key tiles per batch
CC = T // CHUNK        # 8 chunks per batch

# e-slot layout: kt<8 -> 512 wide (both q-chunks), kt>=8 -> 256 (q-chunk 1)
def _slot_off(kt):
    return kt * 512 if kt < 8 else 4096 + (kt - 8) * 256


def _slot_w(kt):
    return 512 if kt < 8 else 256


GMASK_W = 8 * 512 + 8 * 256      # 6144: gathered slots
LOFF = GMASK_W                   # local slots: 4 x 512 wide
MASK_W = GMASK_W + 4 * 512       # 8192
# exp groups: contiguous 1024-col spans of the slot layout
GROUPS = [(0, 2), (2, 2), (4, 2), (6, 2), (8, 4), (12, 4)]
GROUPS_LOC = [(0, 2), (2, 2)]
LOCAL_PASS = False

KT_ELEMS = 6 * 128 * TLOC          # k^T bounce: [6 ct][128 p][512 t]
V_ELEMS = 4 * 128 * C              # v bounce:   [4 tt][128 p][768 c]

LAST_EXEC_NS = None
LAST_RESULTS = None
_CACHE = {}


def _rank_of_chunk(ck):
    return ck if ck < 4 else 7 - ck


def _loc_of_chunk(ck):
    return 0 if ck < 4 else CHUNK


def _build(add_qk_bias, add_proj_bias, add_fc2_bias):
    nc = bacc.Bacc("TRN2", target_bir_lowering=False, debug=False,
                   num_devices=NCORES)

    x_ext = nc.dram_tensor("x", [TLOC, C], F32, kind="ExternalInput")
    wq_ext = nc.dram_tensor("wq", [3, 128, 2, C], FP8, kind="ExternalInput")
    wk_ext = nc.dram_tensor("wk", [3, 128, 2, C], FP8, kind="ExternalInput")
    wv_ext = nc.dram_tensor("wv", [3, 128, 2, C], FP8, kind="ExternalInput")
    wp_ext = nc.dram_tensor("wp", [6, 128, 2, C], FP8, kind="ExternalInput")
    wfc_ext = nc.dram_tensor("wfc", [C, 4 * C], BF16, kind="ExternalInput")
    wfc2_ext = nc.dram_tensor("wfc2", [4 * C, C], BF16, kind="ExternalInput")
    masks_ext = nc.dram_tensor("masks", [128, MASK_W], BF16,
                               kind="ExternalInput")
    bqk_ext = nc.dram_tensor("bqk", [2, C], F32, kind="ExternalInput")
    bfc_ext = nc.dram_tensor("bfc", [4 * C], F32, kind="ExternalInput")
    bout_ext = nc.dram_tensor("bout", [2, C], F32, kind="ExternalInput")
    out_ext = nc.dram_tensor("out", [TLOC, C], F32, kind="ExternalOutput")

    with tile.TileContext(nc) as tc:
        with tc.tile_pool(name="dram", bufs=1, space="DRAM") as dram, \
             tc.tile_pool(name="singles", bufs=1) as singles, \
             tc.tile_pool(name="persist", bufs=1) as persist, \
             tc.tile_pool(name="small", bufs=3) as small:

            KH = KT_ELEMS // 2
            VH = V_ELEMS // 2
            sync_in = dram.tile([128], FP8)
            sync_all = dram.tile([4, 128], FP8)
            kvk_in1 = dram.tile([KH], FP8)
            kvk_all1 = dram.tile([4, KH], FP8)
            kvk_in2 = dram.tile([KH], FP8)
            kvk_all2 = dram.tile([4, KH], FP8)
            kvv_in1 = dram.tile([VH], FP8)
            kvv_all1 = dram.tile([4, VH], FP8)
            kvv_in2 = dram.tile([VH], FP8)
            kvv_all2 = dram.tile([4, VH], FP8)

            ident = singles.tile([128, 128], BF16)
            make_identity(nc, ident)
            eps_sb = singles.tile([128, 1], F32)
            nc.vector.memset(eps_sb, EPS)
            ones_pad = singles.tile([128, 64], F32)
            nc.vector.memset(ones_pad, 0.0)
            nc.vector.memset(ones_pad[0:1, :], 1.0)
            d_sb = singles.tile([128, TLOC], F32)
            nc.vector.memset(d_sb, 1.0)

            x_sb = persist.tile([128, 4, C], F32)     # local x, becomes xmid
            for t in range(4):
                nc.sync.dma_start(out=x_sb[:, t, :],
                                  in_=x_ext[t * 128:(t + 1) * 128, :])
            wp_sb = persist.tile([128, 6, 2, C], FP8)

            bqk_sb = singles.tile([128, 2, 6], F32)
            if add_qk_bias:
                nc.sync.dma_start(
                    out=bqk_sb,
                    in_=bqk_ext.ap().rearrange("b (m p) -> p b m", p=128))
            bfc_sb = singles.tile([128, 24], F32)
            nc.sync.dma_start(
                out=bfc_sb, in_=bfc_ext.ap().rearrange("(m p) -> p m", p=128))
            bout_sb = singles.tile([128, 2, C], F32)
            if add_proj_bias or add_fc2_bias:
                bc = bout_ext.ap()
                nc.sync.dma_start(
                    out=bout_sb,
                    in_=bass.AP(tensor=bc.tensor, offset=bc.offset,
                                ap=[[0, 128], bc.ap[0], bc.ap[1]]))

            masks_sb = persist.tile([128, MASK_W], BF16)

            hT = persist.tile([128, 6, TLOC], BF16)   # h2^T for the MLP
            hT8 = persist.tile([128, 6, TLOC], FP8)   # h^T for QKV (DR pairs)
            qT = persist.tile([128, 12, TLOC], FP8)
            nc.vector.memset(qT[:], 0.0)
            # rows 64:128 must be zeroed: garbage bytes can decode to fp8
            # NaN, and NaN * 0 = NaN in the PE accumulation
            yT_all = persist.tile([128, 12, TLOC], FP8)
            nc.vector.memset(yT_all[64:128, :, :], 0.0)

            def layernorm_to(pool, xt, dst, tagsuf):
                stats = pool.tile([128, 3, 6], F32, tag="st" + tagsuf,
                                  name="st" + tagsuf)
                for sg in range(3):
                    nc.vector.bn_stats(out=stats[:, sg, :],
                                       in_=xt[:, sg * 256:(sg + 1) * 256])
                mv = pool.tile([128, 2], F32, tag="mv" + tagsuf,
                               name="mv" + tagsuf)
                nc.vector.bn_aggr(out=mv, in_=stats)
                nc.scalar.activation(out=mv[:, 1:2], in_=mv[:, 1:2],
                                     func=mybir.ActivationFunctionType.Sqrt,
                                     bias=eps_sb)
                nc.vector.reciprocal(out=mv[:, 1:2], in_=mv[:, 1:2])
                nc.vector.tensor_scalar(out=dst, in0=xt,
                                        scalar1=mv[:, 0:1], scalar2=mv[:, 1:2],
                                        op0=mybir.AluOpType.subtract,
                                        op1=mybir.AluOpType.mult)

            # ---------------- LN1 + transpose + QKV + AGs ----------------
            locp = tc.alloc_tile_pool(name="loc", bufs=1)
            esbp = tc.alloc_tile_pool(name="esb", bufs=4)
            if LOCAL_PASS:
                loc_y = locp.tile([65, 12, TLOC], BF16)
                klocp = tc.alloc_tile_pool(name="kloc", bufs=1)
                kT_bf = klocp.tile([128, 6, TLOC], BF16)
                va_loc = klocp.tile([128, 4, 12 * 65], BF16)
                val4 = va_loc[:].rearrange("p l (h e) -> p l h e", e=65)
                nc.vector.memset(val4[:, :, :, 64:65], 1.0)
            with tc.tile_pool(name="ln", bufs=3) as lnp, \
                 tc.tile_pool(name="tp", bufs=2, space="PSUM") as tpp, \
                 tc.tile_pool(name="qkp", bufs=2, space="PSUM") as qkp, \
                 tc.tile_pool(name="vp", bufs=2, space="PSUM") as vpp, \
                 tc.tile_pool(name="vsb", bufs=1) as vsbp:

                kT = vsbp.tile([128, 6, TLOC], FP8)
                wq_sb = vsbp.tile([128, 3, 2, C], FP8)
                wk_sb = vsbp.tile([128, 3, 2, C], FP8)
                wv_sb = vsbp.tile([128, 3, 2, C], FP8)
                for sb, ext in ((wq_sb, wq_ext), (wk_sb, wk_ext),
                                (wv_sb, wv_ext)):
                    nc.sync.dma_start(
                        out=sb,
                        in_=ext.ap().rearrange("g p l c -> p g l c"))
                # larger, later-needed loads issued after the QKV weights
                # so they don't delay the k matmuls / first collective
                nc.sync.dma_start(out=masks_sb, in_=masks_ext.ap())
                nc.sync.dma_start(
                    out=wp_sb, in_=wp_ext.ap().rearrange("g p l c -> p g l c"))
                for t in range(4):
                    xn = lnp.tile([128, C], BF16, tag="xn")
                    layernorm_to(lnp, x_sb[:, t, :], xn, "1")
                    for ct in range(6):
                        pt = tpp.tile([128, 128], BF16, tag="tp")
                        nc.tensor.transpose(
                            pt, xn[:, ct * 128:(ct + 1) * 128], ident)
                        nc.vector.tensor_copy(
                            hT8[:, ct, t * 128:(t + 1) * 128], pt)

                # k^T first: it feeds the first collective
                def k_mtile(m):
                    ps = qkp.tile([128, TLOC], F32, tag="qk", name="psk")
                    for g in range(3):
                        nc.tensor.matmul(
                            ps, lhsT=wk_sb[:, g, :, m * 128:(m + 1) * 128],
                            rhs=hT8[:, 2 * g:2 * g + 2, :],
                            start=(g == 0), stop=(g == 2), perf_mode=DR)
                    if add_qk_bias:
                        nc.vector.tensor_scalar_add(
                            out=kT[:, m, :], in0=ps,
                            scalar1=bqk_sb[:, 1, m:m + 1])
                        nc.vector.tensor_scalar_add(
                            out=kT_bf[:, m, :], in0=ps,
                            scalar1=bqk_sb[:, 1, m:m + 1])
                    else:
                        nc.vector.tensor_copy(kT[:, m, :], ps)
                        if LOCAL_PASS:
                            nc.vector.tensor_copy(kT_bf[:, m, :], ps)

                for m in range(3):
                    k_mtile(m)
                nc.sync.dma_start(
                    out=kvk_in1[:].rearrange("(ct p t) -> p ct t", p=128,
                                             t=TLOC),
                    in_=kT[:, 0:3, :])
                nc.gpsimd.collective_compute(
                    "AllGather", mybir.AluOpType.bypass,
                    replica_groups=[[0, 1, 2, 3], [4, 5, 6, 7]],
                    ins=[kvk_in1[:].opt()], outs=[kvk_all1[:].opt()])
                for m in range(3, 6):
                    k_mtile(m)
                nc.sync.dma_start(
                    out=kvk_in2[:].rearrange("(ct p t) -> p ct t", p=128,
                                             t=TLOC),
                    in_=kT[:, 3:6, :])
                nc.gpsimd.collective_compute(
                    "AllGather", mybir.AluOpType.bypass,
                    replica_groups=[[0, 1, 2, 3], [4, 5, 6, 7]],
                    ins=[kvk_in2[:].opt()], outs=[kvk_all2[:].opt()])

                v_sb = vsbp.tile([128, 4, C], FP8)
                for tt in range(4):
                    pv = vpp.tile([128, C], F32, tag="v")
                    for g in range(3):
                        nc.tensor.matmul(
                            pv[:, 0:512],
                            lhsT=hT8[:, 2 * g:2 * g + 2,
                                     tt * 128:(tt + 1) * 128],
                            rhs=wv_sb[:, g, :, 0:512],
                            start=(g == 0), stop=(g == 2), perf_mode=DR)
                        nc.tensor.matmul(
                            pv[:, 512:768],
                            lhsT=hT8[:, 2 * g:2 * g + 2,
                                     tt * 128:(tt + 1) * 128],
                            rhs=wv_sb[:, g, :, 512:768],
                            start=(g == 0), stop=(g == 2), perf_mode=DR)
                    nc.vector.tensor_copy(v_sb[:, tt, :], pv)
                    if LOCAL_PASS:
                        nc.vector.tensor_copy(
                            val4[:, tt, :, 0:64],
                            pv.rearrange("p (h e) -> p h e", e=64))
                # v split by token-half: v1 = local chunks 0 (-> kt<8 slots),
                # v2 = local chunks 1 (-> kt>=8), so AV can start after v1.
                nc.sync.dma_start(
                    out=kvv_in1[:].rearrange("(tt p c) -> p tt c", p=128,
                                             c=C),
                    in_=v_sb[:, 0:2, :])
                nc.gpsimd.collective_compute(
                    "AllGather", mybir.AluOpType.bypass,
                    replica_groups=[[0, 1, 2, 3], [4, 5, 6, 7]],
                    ins=[kvv_in1[:].opt()], outs=[kvv_all1[:].opt()])
                nc.sync.dma_start(
                    out=kvv_in2[:].rearrange("(tt p c) -> p tt c", p=128,
                                             c=C),
                    in_=v_sb[:, 2:4, :])
                nc.gpsimd.collective_compute(
                    "AllGather", mybir.AluOpType.bypass,
                    replica_groups=[[0, 1, 2, 3], [4, 5, 6, 7]],
                    ins=[kvv_in2[:].opt()], outs=[kvv_all2[:].opt()])

                for m in range(6):
                    ps = qkp.tile([128, TLOC], F32, tag="qk")
                    for g in range(3):
                        nc.tensor.matmul(
                            ps, lhsT=wq_sb[:, g, :, m * 128:(m + 1) * 128],
                            rhs=hT8[:, 2 * g:2 * g + 2, :],
                            start=(g == 0), stop=(g == 2), perf_mode=DR)
                    for par in range(2):
                        h = 2 * m + par
                        sl = slice(par * 64, par * 64 + 64)
                        if add_qk_bias:
                            nc.vector.tensor_scalar_add(
                                out=qT[sl, h, :], in0=ps[sl, :],
                                scalar1=bqk_sb[sl, 0, m:m + 1])
                        else:
                            nc.vector.tensor_copy(qT[sl, h, :], ps[sl, :])

            # ---- local (pre-AG) attention on own chunks ----
            if not LOCAL_PASS:
                pass
            with tc.tile_pool(name="epl", bufs=2, space="PSUM") as eplp, \
                 tc.tile_pool(name="avl", bufs=2, space="PSUM") as avlp:
                pavl = {}
                lpends = []
                last_loc = {}

                def emit_av_loc(pend):
                    h, e_sb, (g0, gn) = pend
                    for i in range(gn):
                        ktl = g0 + i
                        last_loc["pe"] = nc.tensor.matmul(
                            pavl[h],
                            lhsT=va_loc[:, ktl, h * 65:(h + 1) * 65],
                            rhs=e_sb[:, i * 512:(i + 1) * 512],
                            start=(ktl == 0), stop=(ktl == 3),
                            skip_group_check=True)
                    if g0 + gn == 4:
                        nc.vector.tensor_copy(loc_y[:, h, :], pavl[h])
                        del pavl[h]

                for h in range(12 if LOCAL_PASS else 0):
                    pavl[h] = avlp.tile([65, TLOC], F32, tag="avl",
                                        name=f"pavl_{h}")
                    for (g0, gn) in GROUPS_LOC:
                        pe = eplp.tile([128, 1024], F32, tag="el")
                        for i in range(gn):
                            ktl = g0 + i
                            nc.tensor.matmul(
                                pe[:, i * 512:(i + 1) * 512],
                                lhsT=kT_bf[:, h // 2,
                                           ktl * 128:(ktl + 1) * 128],
                                rhs=qT[:, h, :], start=True, stop=True)
                        e_sb = esbp.tile([128, 1024], BF16, tag="esb")
                        last_loc["act"] = nc.scalar.activation(
                            out=e_sb, in_=pe,
                            func=mybir.ActivationFunctionType.Exp)
                        off = LOFF + g0 * 512
                        last_loc["dve"] = nc.vector.tensor_mul(
                            e_sb, e_sb, masks_sb[:, off:off + 1024])
                        lpends.append((h, e_sb, (g0, gn)))
                        if len(lpends) > 2:
                            emit_av_loc(lpends.pop(0))
                for pend in lpends:
                    emit_av_loc(pend)
                lpends = []
            if LOCAL_PASS:
                klocp.release()

            # ---------------- attention ----------------
            with tc.tile_pool(name="kch", bufs=1) as kchp, \
                 tc.tile_pool(name="vch", bufs=1) as vchp, \
                 tc.tile_pool(name="vaug", bufs=1) as vaugp, \
                 tc.tile_pool(name="ep", bufs=2, space="PSUM") as epp, \
                 tc.tile_pool(name="avp", bufs=2, space="PSUM") as avpp, \
                 tc.tile_pool(name="bcp", bufs=1, space="PSUM") as bcpp:

                # gathered k half: [r][3 ct][128][512]; v half: [r][4 lt][128][384]
                k_ch = kchp.tile([128, 4, 6, TLOC], FP8)
                v_ch = vchp.tile([128, 4, 4, C], FP8)
                v_aug = vaugp.tile([128, NKT, 12 * 65], BF16)
                va4 = v_aug[:].rearrange("p kt (h e) -> p kt h e", e=65)
                ms = nc.vector.memset(va4[:, :, :, 64:65], 1.0)
                if "dve" in last_loc:
                    add_dep_helper(ms.ins, last_loc["dve"].ins,
                                   sync=True, reason="local-b4-vaones")
                for hh, (kvk_a, kvv_a) in enumerate(
                        ((kvk_all1, kvv_all1), (kvk_all2, kvv_all2))):
                    for r in range(4):
                        nc.sync.dma_start(
                            out=k_ch[:, r, 3 * hh:3 * hh + 3, :],
                            in_=kvk_a[r].rearrange("(ct p t) -> p ct t",
                                                   p=128, t=TLOC))
                    for r in range(4):
                        nc.sync.dma_start(
                            out=v_ch[:, r, 2 * hh:2 * hh + 2, :],
                            in_=kvv_a[r].rearrange("(tt p c) -> p tt c",
                                                   p=128, c=C))
                        kt0 = 2 * r if hh == 0 else 14 - 2 * r
                        vsrc = v_ch[:, r, 2 * hh:2 * hh + 2, :].rearrange(
                            "p l (h e) -> p l h e", e=64)
                        vc = nc.vector.tensor_copy(
                            va4[:, kt0:kt0 + 2, :, 0:64], vsrc)
                        if "dve" in last_loc:
                            add_dep_helper(vc.ins, last_loc["dve"].ins,
                                           sync=True,
                                           reason="local-b4-va4")

                def k_ap_of(kt, h):
                    ck = kt // 2
                    r = _rank_of_chunk(ck)
                    loc = _loc_of_chunk(ck) + (kt % 2) * 128
                    return k_ch[:, r, h // 2, loc:loc + 128]

                def finalize_head(h, pav):
                    if LOCAL_PASS:
                        sum_sb = small.tile([65, TLOC], F32, tag="ssb",
                                            name="ssb")
                        nc.vector.tensor_add(sum_sb, loc_y[:, h, :], pav)
                    else:
                        sum_sb = pav
                    nc.vector.tensor_copy(d_sb[0:1, :], sum_sb[64:65, :])
                    pb = bcpp.tile([64, TLOC], F32, tag="bc", name="pbc")
                    nc.tensor.matmul(pb, lhsT=ones_pad, rhs=d_sb,
                                     start=True, stop=True)
                    b_sb = small.tile([64, TLOC], F32, tag="bsb", name="bsb")
                    nc.vector.reciprocal_approx_fast(out=b_sb, in_=pb)
                    nc.vector.tensor_mul(yT_all[0:64, h, :], sum_sb[0:64, :],
                                         b_sb)

                def emit_av(pend):
                    h, e_sb, (g0, gn) = pend
                    pav = pavs[h]
                    off0 = _slot_off(g0)
                    for i in range(gn):
                        kt = g0 + i
                        w = _slot_w(kt)
                        so = _slot_off(kt) - off0
                        out = pav if w == 512 else pav[:, CHUNK:TLOC]
                        nc.tensor.matmul(
                            out, lhsT=v_aug[:, kt, h * 65:(h + 1) * 65],
                            rhs=e_sb[:, so:so + w],
                            start=(kt == 0), stop=(kt == NKT - 1),
                            skip_group_check=True)

                pavs = {}
                pends = []
                for h in range(12):
                    q_full = qT[:, h, :]
                    q_c1 = qT[:, h, CHUNK:TLOC]
                    pavs[h] = avpp.tile([65, TLOC], F32, tag="av",
                                        name=f"pav_{h}")
                    for (g0, gn) in GROUPS:
                        pe = epp.tile([128, 1024], F32, tag="e")
                        off0 = _slot_off(g0)
                        for i in range(gn):
                            kt = g0 + i
                            w = _slot_w(kt)
                            so = _slot_off(kt) - off0
                            mm = nc.tensor.matmul(
                                pe[:, so:so + w], lhsT=k_ap_of(kt, h),
                                rhs=(q_full if w == 512 else q_c1),
                                start=True, stop=True)
                            if "pe" in last_loc:
                                add_dep_helper(mm.ins, last_loc.pop("pe").ins,
                                               sync=True,
                                               reason="local-b4-gathered")
                        e_sb = esbp.tile([128, 1024], BF16, tag="esb")
                        ex = nc.scalar.activation(
                            out=e_sb, in_=pe,
                            func=mybir.ActivationFunctionType.Exp)
                        if "act" in last_loc:
                            add_dep_helper(ex.ins, last_loc.pop("act").ins,
                                           sync=True,
                                           reason="local-b4-gathered")
                        mk = nc.vector.tensor_mul(
                            e_sb, e_sb, masks_sb[:, off0:off0 + 1024])
                        last_loc.pop("dve", None)

                        pends.append((h, e_sb, (g0, gn)))
                        if len(pends) > 4:
                            pend = pends.pop(0)
                            emit_av(pend)
                            if pend[2][0] + pend[2][1] == NKT:
                                finalize_head(pend[0], pavs[pend[0]])
                                del pavs[pend[0]]
                for pend in pends:
                    emit_av(pend)
                    if pend[2][0] + pend[2][1] == NKT:
                        finalize_head(pend[0], pavs[pend[0]])
                        del pavs[pend[0]]
                pends = []

            esbp.release()
            locp.release()

            # ---------------- proj + residual + LN2 ----------------
            with tc.tile_pool(name="pp", bufs=2, space="PSUM") as ppp, \
                 tc.tile_pool(name="ln2", bufs=3) as ln2p, \
                 tc.tile_pool(name="tp2", bufs=2, space="PSUM") as tpp2:

                xn2s = []
                for t in range(4):
                    pp = ppp.tile([128, C], F32, tag="pp")
                    for g in range(6):
                        y_ap = yT_all[:, 2 * g:2 * g + 2,
                                      t * 128:(t + 1) * 128]
                        nc.tensor.matmul(pp[:, 0:512], lhsT=y_ap,
                                         rhs=wp_sb[:, g, :, 0:512],
                                         start=(g == 0), stop=(g == 5),
                                         perf_mode=DR)
                        nc.tensor.matmul(pp[:, 512:768], lhsT=y_ap,
                                         rhs=wp_sb[:, g, :, 512:768],
                                         start=(g == 0), stop=(g == 5),
                                         perf_mode=DR)
                    nc.vector.tensor_add(x_sb[:, t, :], x_sb[:, t, :], pp)
                    if add_proj_bias:
                        nc.vector.tensor_add(x_sb[:, t, :], x_sb[:, t, :],
                                             bout_sb[:, 0, :])
                    xn2 = ln2p.tile([128, C], BF16, tag="xn2", name="xn2")
                    layernorm_to(ln2p, x_sb[:, t, :], xn2, "2")
                    xn2s.append(xn2)
                for t in range(4):
                    for ct in range(6):
                        pt = tpp2.tile([128, 128], BF16, tag="tp2")
                        nc.tensor.transpose(
                            pt, xn2s[t][:, ct * 128:(ct + 1) * 128], ident)
                        nc.vector.tensor_copy(
                            hT[:, ct, t * 128:(t + 1) * 128], pt)

            # ---------------- MLP ----------------
            with tc.tile_pool(name="mlp", bufs=1) as mlpp, \
                 tc.tile_pool(name="wfc", bufs=6) as wfcp, \
                 tc.tile_pool(name="wfc2", bufs=6) as wfc2p, \
                 tc.tile_pool(name="osb", bufs=3) as osbp:

                gT = mlpp.tile([128, 24, TLOC], BF16)
                wfc_t = wfc_ext.ap().rearrange("(k p) n -> p k n", p=128)
                with tc.tile_pool(name="fcp", bufs=2, space="PSUM") as fcpp:
                    for m in range(24):
                        wt = wfcp.tile([128, 6, 128], BF16, tag="wfc")
                        nc.sync.dma_start(
                            out=wt, in_=wfc_t[:, :, m * 128:(m + 1) * 128])
                        pf = fcpp.tile([128, TLOC], F32, tag="fc")
                        for k in range(6):
                            nc.tensor.matmul(pf, lhsT=wt[:, k, :],
                                             rhs=hT[:, k, :],
                                             start=(k == 0), stop=(k == 5))
                        nc.scalar.activation(
                            out=gT[:, m, :], in_=pf,
                            func=mybir.ActivationFunctionType.Gelu_apprx_tanh,
                            bias=bfc_sb[:, m:m + 1])

                wfc2_t = wfc2_ext.ap().rearrange("(k p) n -> k p n", p=128)
                with tc.tile_pool(name="f2p", bufs=1, space="PSUM") as f2pp:
                    pf2s = [f2pp.tile([128, C], F32, tag=f"f2_{t}",
                                      name=f"pf2_{t}")
                            for t in range(4)]
                    for k in range(24):
                        wt2 = wfc2p.tile([128, C], BF16, tag="wfc2")
                        nc.sync.dma_start(out=wt2, in_=wfc2_t[k])
                        for t in range(4):
                            nc.tensor.matmul(
                                pf2s[t][:, 0:512],
                                lhsT=gT[:, k, t * 128:(t + 1) * 128],
                                rhs=wt2[:, 0:512],
                                start=(k == 0), stop=(k == 23))
                            nc.tensor.matmul(
                                pf2s[t][:, 512:768],
                                lhsT=gT[:, k, t * 128:(t + 1) * 128],
                                rhs=wt2[:, 512:768],
                                start=(k == 0), stop=(k == 23))
                    for t in range(4):
                        o_sb = osbp.tile([128, C], F32, tag="osb", name="osb")
                        nc.vector.tensor_add(o_sb, x_sb[:, t, :], pf2s[t])
                        if add_fc2_bias:
                            nc.vector.tensor_add(o_sb, o_sb, bout_sb[:, 1, :])
                        nc.sync.dma_start(
                            out=out_ext[t * 128:(t + 1) * 128, :], in_=o_sb)

    nc.compile()
    return nc


def _preprocess(inputs):
    f = lambda k: np.asarray(inputs[k], np.float32)
    x = f("x"); w_attn = f("w_attn"); b_attn = f("b_attn")
    w_proj = f("w_proj"); b_proj = f("b_proj")
    w_fc = f("w_fc"); b_fc = f("b_fc"); w_fc2 = f("w_fc2"); b_fc2 = f("b_fc2")
    ln1_g = f("ln1_g"); ln1_b = f("ln1_b"); ln2_g = f("ln2_g"); ln2_b = f("ln2_b")

    w_attn_eff = ln1_g[:, None] * w_attn
    b_attn_eff = b_attn + ln1_b @ w_attn
    s = 1.0 / np.sqrt(HD)
    w_q = w_attn_eff[:, 0:C] * s
    w_k = w_attn_eff[:, C:2 * C]
    w_v = w_attn_eff[:, 2 * C:3 * C]
    b_q = b_attn_eff[0:C] * s
    b_k = b_attn_eff[C:2 * C]
    b_v = b_attn_eff[2 * C:3 * C]
    b_proj_eff = b_proj + b_v @ w_proj
    w_fc_eff = ln2_g[:, None] * w_fc
    b_fc_eff = b_fc + ln2_b @ w_fc

    def pack_dr(w):
        # [768, N] -> [3, 128, 2, N]: lhsT slice [kp][:, :, m0:m1]
        return np.ascontiguousarray(
            w.reshape(3, 2, 128, -1).transpose(0, 2, 1, 3).astype(E4))

    wq8 = pack_dr(w_q)
    wk8 = pack_dr(w_k)
    wv8 = pack_dr(w_v)
    # proj: head-pair DR groups; partitions 0:64 = head dims, 64:128 zero
    wp_pad = np.zeros((6, 2, 128, C), np.float32)
    wp_pad[:, :, 0:64, :] = w_proj.reshape(6, 2, 64, C)
    wp8 = np.ascontiguousarray(wp_pad.transpose(0, 2, 1, 3).astype(E4))
    wfc16 = np.ascontiguousarray(w_fc_eff.astype(BF))
    wfc216 = np.ascontiguousarray(w_fc2.astype(BF))

    bqk = np.stack([b_q, b_k]).astype(np.float32)
    bout = np.stack([b_proj_eff, b_fc2]).astype(np.float32)

    flags = (bool(np.any(bqk != 0)), bool(np.any(b_proj_eff != 0)),
             bool(np.any(b_fc2 != 0)))

    # mask slab [128, 8192] per core group j; kt<8 slots cover both q-chunks
    kpos = np.arange(128)
    qpos = np.arange(CHUNK)
    masks = np.zeros((4, 128, MASK_W), np.float32)
    for j in range(4):
        own = ({2 * j, 2 * j + 1, 14 - 2 * j, 15 - 2 * j} if LOCAL_PASS
               else set())
        for kt in range(NKT):
            if kt in own:
                continue  # handled by the local pre-AG pass
            gk = kt * 128 + kpos[:, None]
            off = _slot_off(kt)
            if kt < 8:
                gq0 = j * CHUNK + qpos[None, :]
                gq1 = (7 - j) * CHUNK + qpos[None, :]
                masks[j, :, off:off + 256] = (gq0 >= gk)
                masks[j, :, off + 256:off + 512] = (gq1 >= gk)
            else:
                gq1 = (7 - j) * CHUNK + qpos[None, :]
                masks[j, :, off:off + 256] = (gq1 >= gk)
        # local slots (position-independent): ktl 0,1 = own chunk0 diag vs
        # q0, ones vs q1; ktl 2,3 = zeros vs q0, own chunk1 diag vs q1
        for ktl in range(4):
            off = LOFF + ktl * 512
            tri = (qpos[None, :] >= (ktl % 2) * 128 + kpos[:, None])
            if ktl < 2:
                masks[j, :, off:off + 256] = tri
                masks[j, :, off + 256:off + 512] = 1.0
            else:
                masks[j, :, off + 256:off + 512] = tri
    masks16 = masks.astype(BF)

    in_maps = []
    for c in range(NCORES):
        b, j = c // 4, c % 4
        x_loc = np.concatenate(
            [x[b, j * CHUNK:(j + 1) * CHUNK],
             x[b, (7 - j) * CHUNK:(8 - j) * CHUNK]]).astype(np.float32)
        in_maps.append({
            "x": np.ascontiguousarray(x_loc),
            "wq": wq8, "wk": wk8, "wv": wv8, "wp": wp8,
            "wfc": wfc16, "wfc2": wfc216,
            "masks": np.ascontiguousarray(masks16[j]),
            "bqk": bqk, "bfc": b_fc_eff.astype(np.float32), "bout": bout,
        })
    return in_maps, flags


def kernel(**inputs):
    global LAST_EXEC_NS, LAST_RESULTS
    in_maps, flags = _preprocess(inputs)
    if flags not in _CACHE:
        _CACHE[flags] = _build(*flags)
    nc = _CACHE[flags]
    trace = bool(os.environ.get("BASS_KERNEL_TRACE"))
    res = run_bass_kernel_spmd(nc, in_maps, core_ids=list(range(NCORES)),
                               trace=trace)
    LAST_EXEC_NS = res.exec_time_ns
    LAST_RESULTS = res
    out = np.empty((B, T, C), np.float32)
    for c in range(NCORES):
        b, j = c // 4, c % 4
        o = res.results[c]["out"]
        out[b, j * CHUNK:(j + 1) * CHUNK] = o[0:CHUNK]
        out[b, (7 - j) * CHUNK:(8 - j) * CHUNK] = o[CHUNK:TLOC]
    return out

